# revision 23
# baseline (speedup 1.0000x reference)
"""Trainium2 Bass kernel v3h for the dense_cnn problem:

    t1 = conv1x1(x, w1); t2 = gelu(t1)
    t5 = dwconv5x5(t2, w5, pad=2)
    t6 = dwconv7x7_dil3(t5, w6, pad=9)
    t7 = conv1x1(t6, w7); t8 = t7 * t2; t9 = conv1x1(t8, w9)
    out = x + t9

Sharding: data-parallel over batch N=32 across 8 cores (4 samples/core).

Design (measured-cost driven):
  - PE matmuls sustain ~0.42ns/row back-to-back (LDWEIGHTS hidden), so the
    1x1 convs and dw7 run as diag matmuls.
  - dw5 runs on DVE/ACT/Pool chains (two independent tap-subchains per
    group); dw7 entirely on the PE as fp8e4 DoubleRow tap-pair diagonal
    matmuls (2 taps per 448-row pass).
  - v3h: the fp8 pair-diagonal weights for dw7 are precomputed on the
    HOST and DMA'd in (v3 built them with ~150 DVE ops serialized ahead
    of the first dw7 matmul).
  - PSUM allocated as 4-bank "wave" tiles [128, 4x512]; matmuls write
    448-elem chunks at 512 strides; evictions/elementwise consumers read
    the whole wave with one strided instruction.
  - x is cast to f16 on the host; one DMA in, reused by stage B and the
    residual add. Output DMA'd in 1792-elem waves.
"""

import numpy as np
import ml_dtypes

import concourse.bass as bass
import concourse.mybir as mybir
from concourse.tile import TileContext
from concourse.bass_utils import run_bass_kernel_spmd

# ---------------------------------------------------------------------------
# Workaround: this walrus build rejects >N sem waits on the TileContext tail
# drain ("Too many sync wait commands"). Split them one-per-drain.
from concourse.vector_clock import ScopedClock, VectorClock


def _drain_and_barrier_split(self, tick_clock, wait_clock):
    vc = tick_clock.global_clock
    for proc in range(len(vc)):
        tick = vc[proc]
        if tick <= 0:
            continue
        d = self.nc.sync.drain()
        req = ScopedClock({None: VectorClock([0] * len(vc))})
        req.require_at_least(None, proc, tick)
        wait_clock.add_sem_waits(d.ins, req)
    self.nc.all_engine_barrier()
    assert self.sems is not None
    popped = self.nc._tile_sem_poison_stack.pop()
    assert popped is self._sem_poison
    self.nc.clear_and_free_semaphores(list(self.sems.allocated().values()))
    self.nc.all_engine_barrier()


TileContext._drain_and_barrier = _drain_and_barrier_split

# This walrus build also rejects >1 sem wait on regular engine instructions.
# Post-process the serialized BIR: hoist excess waits onto same-engine NoOps
# inserted right before the instruction.
import json as _json

_orig_to_json_bytes = bass.Bass.to_json_bytes


def _to_json_bytes_split_waits(self):
    d = _json.loads(_orig_to_json_bytes(self))
    ctr = 0
    for fn in d.get("functions", []):
        for blk in fn.get("blocks", []):
            insts = blk.get("instructions", [])
            out = []
            for inst in insts:
                si = inst.get("sync_info")
                waits = (si or {}).get("on_wait") or []
                if len(waits) > 1:
                    for w in waits[:-1]:
                        out.append({
                            "debug": inst.get("debug", 0),
                            "engine": inst["engine"],
                            "ins": [],
                            "outs": [],
                            "name": f"{inst['name']}_hw{ctr}",
                            "opcode": "NoOp",
                            "sync_info": {"on_wait": [w], "on_update": []},
                        })
                        ctr += 1
                    si["on_wait"] = waits[-1:]
                out.append(inst)
            blk["instructions"] = out
    return _json.dumps(d).encode()


bass.Bass.to_json_bytes = _to_json_bytes_split_waits
# ---------------------------------------------------------------------------

F16 = mybir.dt.float16
F32 = mybir.dt.float32
AF = mybir.ActivationFunctionType
OP = mybir.AluOpType

N_CORES = 8
NS = 4              # samples per core
C, H, W = 384, 56, 56
G = 3               # channel groups of 128
HW = H * W          # 3136
W5P = 60            # t2 padded width/height (pad 2)
W7P = 74            # t5 padded width/height (pad 9)
W7PP = 80           # t5pad8 row pitch (32B-aligned partition pitch for PE fp8)
CH_ROWS = 8         # output rows per PSUM chunk
BANK = 512          # f32 elems per PSUM bank
CHF = CH_ROWS * W   # 448 elems per chunk

# dw5 runs entirely on DVE/ACT/Pool chains; dw7 entirely on the PE as
# fp8e4 DoubleRow tap-pair diagonal matmuls (2 taps per 448-row pass).
# Fraction of chain tap multiplies farmed to ACT (i%5 < ACT_OF_5).
ACT_OF_5 = 3
F8 = mybir.dt.float8e4
NP_F8 = ml_dtypes.float8_e4m3
PM = mybir.MatmulPerfMode

DW5_TAPS = [(dy, dx) for dy in range(5) for dx in range(5)]
DW7_TAPS = [(jy, jx) for jy in range(7) for jx in range(7)]
# dw7 tap pairs for DoubleRow (odd count: last pairs with a zero diagonal)
DW7_PAIRS = [(DW7_TAPS[2 * i], DW7_TAPS[2 * i + 1]) for i in range(24)] + [
    (DW7_TAPS[48], None)
]


def _chunks_of(rows):
    """Split `rows` output rows into PSUM chunks of <=8 rows."""
    out = []
    r = 0
    while r < rows:
        n = min(CH_ROWS, rows - r)
        out.append((r, n))
        r += n
    return out


def _waves_of(chunks):
    """Group chunk list into waves of up to 4 (one 4-bank PSUM tile)."""
    return [chunks[i : i + 4] for i in range(0, len(chunks), 4)]


def _build_program():
    nc = bass.Bass("TRN2", target_bir_lowering=False, debug=False)

    x_d = nc.dram_tensor("x16", [NS, G, 128, HW], F16, kind="ExternalInput")
    w1T_d = nc.dram_tensor("w1T", [G, 128, C], F16, kind="ExternalInput")
    w7T_d = nc.dram_tensor("w7T", [G, 128, C], F16, kind="ExternalInput")
    w9T_d = nc.dram_tensor("w9T", [G, 128, C], F16, kind="ExternalInput")
    w5t_d = nc.dram_tensor("w5t", [G, 128, 25], F32, kind="ExternalInput")
    dp6_d = nc.dram_tensor("dp6", [G, 25, 128, 256], F8, kind="ExternalInput")
    o_d = nc.dram_tensor("out", [NS, G, 128, HW], F32, kind="ExternalOutput")

    with TileContext(nc) as tc:
        with (
            tc.tile_pool(name="const", bufs=1) as const,
            tc.tile_pool(name="big16", bufs=6) as big16,
            tc.tile_pool(name="pads", bufs=1) as pads,
            tc.tile_pool(name="xload", bufs=3) as xload_p,
            tc.tile_pool(name="dve", bufs=1) as dve_p,
            tc.tile_pool(name="small", bufs=1) as small_p,
            tc.tile_pool(name="psum", bufs=2, space="PSUM") as pp,
        ):
            # ---- constants -------------------------------------------------
            w1T = [const.tile([128, C], F16, name=f"w1T{k}") for k in range(G)]
            w7T = [const.tile([128, C], F16, name=f"w7T{k}") for k in range(G)]
            w9T = [const.tile([128, C], F16, name=f"w9T{k}") for k in range(G)]
            w5t = [const.tile([128, 25], F32, name=f"w5t{g}") for g in range(G)]
            dpair6 = {}
            for k in range(G):
                nc.sync.dma_start(out=w1T[k][:], in_=w1T_d.ap()[k])
                nc.sync.dma_start(out=w7T[k][:], in_=w7T_d.ap()[k])
                nc.sync.dma_start(out=w9T[k][:], in_=w9T_d.ap()[k])
                nc.sync.dma_start(out=w5t[k][:], in_=w5t_d.ap()[k])
                for pi in range(len(DW7_PAIRS)):
                    t = const.tile([128, 2, 128], F8, name=f"dp6_{k}_{pi}")
                    nc.sync.dma_start(
                        out=t.rearrange("p a b -> p (a b)")[:],
                        in_=dp6_d.ap()[k, pi],
                    )
                    dpair6[(k, pi)] = t

            # ---- padded scratch (zero margins persist across samples) ------
            t2pad = [pads.tile([128, W5P * W5P], F16, name=f"t2p{g}") for g in range(G)]
            t5pad = [pads.tile([128, W7P * W7PP], F8, name=f"t5p{g}") for g in range(G)]
            for g in range(G):
                nc.gpsimd.memset(t2pad[g][:], 0.0)
                nc.gpsimd.memset(t5pad[g][:], 0.0)
            t2p3 = [t.rearrange("p (h w) -> p h w", w=W5P) for t in t2pad]
            t5p3 = [t.rearrange("p (h w) -> p h w", w=W7PP) for t in t5pad]

            env = dict(locals())
            for n in range(NS):
                _emit_sample(nc, env, n)
    return nc


def _psum_wave(env, n, tag, nch):
    """Allocate a PSUM tile of `nch` bank-aligned chunks."""
    pp = env["pp"]
    return pp.tile([128, nch, BANK], F32, name=f"pw{tag}{n}",
                   tag=f"pw{nch}", bufs=1)


import bass_rust as _br


def _dw_chain(nc, env, n, g, taps, wtile, ksz, src3, rows, r_start, dil,
              dst_view, stage, add_eng):
    """Two independent tap-subchains (A: even-indexed taps, B: odd) over rows
    [r_start, r_start+rows); final merge add writes dst_view. Per-tap mul on
    ACT or DVE; adds on `add_eng`."""
    dve_p = env["dve_p"]
    nel = rows * W
    tagsz = "h" if rows <= 28 else ""
    accs = []
    for h in range(2):
        a = dve_p.tile([128, nel], F16, name=f"acc{stage}{n}{g}{r_start}{h}",
                       tag=f"acc{tagsz}", bufs=4)
        accs.append(a.rearrange("p (h w) -> p h w", w=W))
    ntap = len(taps)
    started = [False, False]
    for i, tap in enumerate(taps):
        h = i % 2
        acc3 = accs[h]
        ty, tx = tap[0] * dil, tap[1] * dil
        src = src3[:, r_start + ty : r_start + ty + rows, tx : tx + W]
        sc = wtile[:, ksz * tap[0] + tap[1] : ksz * tap[0] + tap[1] + 1]
        if not started[h]:
            nc.vector.tensor_scalar_mul(acc3[:], src, sc)
            started[h] = True
            continue
        tmp = dve_p.tile([128, nel], F16, name=f"tmp{stage}{n}{g}{r_start}_{i}",
                         tag=f"tmp{tagsz}", bufs=4)
        tmp3 = tmp.rearrange("p (h w) -> p h w", w=W)
        if i % 5 < ACT_OF_5:
            nc.scalar.activation(tmp3[:], src, AF.Copy, scale=sc)
        else:
            nc.vector.tensor_scalar_mul(tmp3[:], src, sc)
        add_eng.tensor_tensor(acc3[:], acc3[:], tmp3[:], OP.add)
    add_eng.tensor_tensor(dst_view, accs[0][:], accs[1][:], OP.add)


def _pair_rhs(t5pad_g, pair, r0, nr):
    """Build the [128, 2, nr, 56] DoubleRow rhs AP for a dw7 tap pair."""
    ta, tb = pair
    off_a = (r0 + 3 * ta[0]) * W7PP + 3 * ta[1]
    if tb is None:
        delta = -3  # in-bounds dummy read, zero diagonal kills it
    else:
        off_b = (r0 + 3 * tb[0]) * W7PP + 3 * tb[1]
        delta = off_b - off_a
    base = t5pad_g[:, off_a : off_a + 1]
    ap = base.copy()
    ap.ap = _br.VecI64Pair(
        [[W7P * W7PP, 128], [delta, 2], [W7PP, nr], [1, W]]
    )
    return ap


def _dw7_pe(nc, env, n, g, dpair6, t5pad_g, t6g3):
    """dw7 entirely on PE: fp8 DoubleRow pair matmuls over all 7 chunks."""
    for wi, wave in enumerate(_waves_of(_chunks_of(H))):
        pw = _psum_wave(env, n, f"D{g}{wi}", len(wave))
        for pi in range(len(DW7_PAIRS)):
            for ci, (r0, nr) in enumerate(wave):
                nc.tensor.matmul(
                    pw[:, ci : ci + 1, 0 : nr * W],
                    dpair6[(g, pi)][:],
                    _pair_rhs(t5pad_g, DW7_PAIRS[pi], r0, nr),
                    start=(pi == 0),
                    stop=(pi == len(DW7_PAIRS) - 1),
                    perf_mode=PM.DoubleRow,
                )
        r0 = wave[0][0]
        rows = sum(nr for _, nr in wave)
        in_ap = pw[:, 0 : len(wave), 0 : CHF].rearrange(
            "p c (r w) -> p c r w", w=W
        )
        out_ap = t6g3[:, r0 : r0 + rows, :].rearrange(
            "p (c r) w -> p c r w", r=CH_ROWS
        )
        nc.scalar.activation(out_ap, in_ap, AF.Copy)


def _emit_sample(nc, env, n):
    x_d, o_d = env["x_d"], env["o_d"]
    w1T, w7T, w9T = env["w1T"], env["w7T"], env["w9T"]
    w5t = env["w5t"]
    dpair6, t5pad = env["dpair6"], env["t5pad"]
    t2p3, t5p3 = env["t2p3"], env["t5p3"]
    big16, xload_p, small_p = env["big16"], env["xload_p"], env["small_p"]

    ALL_CH = _chunks_of(H)  # 7 full chunks

    # A) load x16
    x16 = []
    for g in range(G):
        xl = xload_p.tile([128, HW], F16, name=f"xl{n}{g}", tag="xl")
        nc.sync.dma_start(out=xl[:], in_=x_d.ap()[n, g])
        x16.append(xl)

    # B) t1 = w1 @ x16 ; t2 = gelu(t1) -> t2pad interior
    for m in range(G):
        for wi, wave in enumerate(_waves_of(ALL_CH)):
            pw = _psum_wave(env, n, f"B{m}{wi}", len(wave))
            for k in range(G):
                for ci, (r0, nr) in enumerate(wave):
                    nc.tensor.matmul(
                        pw[:, ci : ci + 1, 0 : nr * W],
                        w1T[k][:, 128 * m : 128 * (m + 1)],
                        x16[k][:, W * r0 : W * (r0 + nr)],
                        start=(k == 0),
                        stop=(k == G - 1),
                    )
            r0 = wave[0][0]
            rows = sum(nr for _, nr in wave)
            in_ap = pw[:, 0 : len(wave), 0 : CHF].rearrange(
                "p c (r w) -> p c r w", w=W
            )
            out_ap = t2p3[m][:, 2 + r0 : 2 + r0 + rows, 2 : 2 + W].rearrange(
                "p (c r) w -> p c r w", r=CH_ROWS
            )
            nc.scalar.activation(out_ap, in_ap, AF.Gelu)

    # C) t5 = dw5(t2) -> t5pad interior (fp8), no PE:
    #    g0, g1 full-image DVE chains; g2 split half DVE / half Pool.
    for g in range(2):
        _dw_chain(
            nc, env, n, g, DW5_TAPS, w5t[g], 5, t2p3[g], H, 0, 1,
            t5p3[g][:, 9 : 9 + H, 9 : 9 + W], "c", nc.vector,
        )
    HD = 42
    _dw_chain(
        nc, env, n, 2, DW5_TAPS, w5t[2], 5, t2p3[2], HD, 0, 1,
        t5p3[2][:, 9 : 9 + HD, 9 : 9 + W], "c", nc.vector,
    )
    _dw_chain(
        nc, env, n, 2, DW5_TAPS, w5t[2], 5, t2p3[2], H - HD, HD, 1,
        t5p3[2][:, 9 + HD : 9 + H, 9 : 9 + W], "cp", nc.gpsimd,
    )

    # D) t6 = dw7_dil3(t5) -> t6 (flat f16), entirely on PE via fp8 pairs
    t6 = []
    for g in range(G):
        t6g = big16.tile([128, HW], F16, name=f"t6_{n}_{g}", tag="b16")
        t6g3 = t6g.rearrange("p (h w) -> p h w", w=W)
        _dw7_pe(nc, env, n, g, dpair6, t5pad[g], t6g3)
        t6.append(t6g)

    # E) t7 = w7 @ t6 ; t8 = t7 * t2 (DVE tt from PSUM wave)
    t8 = []
    for m in range(G):
        t8m = big16.tile([128, HW], F16, name=f"t8_{n}_{m}", tag="b16")
        t8m3 = t8m.rearrange("p (h w) -> p h w", w=W)
        for wi, wave in enumerate(_waves_of(ALL_CH)):
            pw = _psum_wave(env, n, f"E{m}{wi}", len(wave))
            for k in range(G):
                for ci, (r0, nr) in enumerate(wave):
                    nc.tensor.matmul(
                        pw[:, ci : ci + 1, 0 : nr * W],
                        w7T[k][:, 128 * m : 128 * (m + 1)],
                        t6[k][:, W * r0 : W * (r0 + nr)],
                        start=(k == 0),
                        stop=(k == G - 1),
                    )
            r0 = wave[0][0]
            rows = sum(nr for _, nr in wave)
            ps_ap = pw[:, 0 : len(wave), 0 : CHF].rearrange(
                "p c (r w) -> p c r w", w=W
            )
            nc.vector.tensor_tensor(
                t8m3[:, r0 : r0 + rows, :].rearrange(
                    "p (c r) w -> p c r w", r=CH_ROWS
                ),
                ps_ap,
                t2p3[m][:, 2 + r0 : 2 + r0 + rows, 2 : 2 + W].rearrange(
                    "p (c r) w -> p c r w", r=CH_ROWS
                ),
                OP.mult,
            )
        t8.append(t8m)

    # F) t9 = w9 @ t8 ; out = x16 + t9 (DVE from PSUM wave) ; DMA out
    for m in range(G):
        for wi, wave in enumerate(_waves_of(ALL_CH)):
            pw = _psum_wave(env, n, f"F{m}{wi}", len(wave))
            for k in range(G):
                for ci, (r0, nr) in enumerate(wave):
                    nc.tensor.matmul(
                        pw[:, ci : ci + 1, 0 : nr * W],
                        w9T[k][:, 128 * m : 128 * (m + 1)],
                        t8[k][:, W * r0 : W * (r0 + nr)],
                        start=(k == 0),
                        stop=(k == G - 1),
                    )
            r0 = wave[0][0]
            nf = sum(nr for _, nr in wave) * W
            ost = small_p.tile([128, nf], F32, name=f"os{n}{m}{wi}",
                               tag=f"ost{wi}", bufs=2 if wi == 0 else 1)
            ps_ap = pw[:, 0 : len(wave), 0 : CHF]
            nc.vector.tensor_tensor(
                ost.rearrange("p (c f) -> p c f", f=CHF),
                ps_ap,
                x16[m][:, W * r0 : W * r0 + nf].rearrange(
                    "p (c f) -> p c f", f=CHF
                ),
                OP.add,
            )
            nc.sync.dma_start(
                out=o_d.ap()[n, m, :, W * r0 : W * r0 + nf], in_=ost[:]
            )


_NC_CACHE = None


def _get_nc():
    global _NC_CACHE
    if _NC_CACHE is None:
        _NC_CACHE = _build_program()
    return _NC_CACHE


def _prep_shared_inputs(w1, w5, w6, w7, w9):
    def lhsT(w):
        return (
            np.ascontiguousarray(np.asarray(w, np.float32).T)
            .astype(np.float16)
            .reshape(G, 128, C)
        )

    idx = np.arange(128)
    w6f = np.asarray(w6, np.float32).reshape(C, 49)
    dp6 = np.zeros((G, 25, 128, 256), NP_F8)
    for g in range(G):
        for pi, (ta, tb) in enumerate(DW7_PAIRS):
            blk = np.zeros((128, 256), np.float32)
            blk[idx, idx] = w6f[g * 128 : (g + 1) * 128, 7 * ta[0] + ta[1]]
            if tb is not None:
                blk[idx, 128 + idx] = w6f[g * 128 : (g + 1) * 128, 7 * tb[0] + tb[1]]
            dp6[g, pi] = blk.astype(NP_F8)

    return {
        "w1T": lhsT(w1),
        "w7T": lhsT(w7),
        "w9T": lhsT(w9),
        "w5t": np.asarray(w5, np.float32).reshape(C, 25).reshape(G, 128, 25),
        "dp6": dp6,
    }


def _make_in_maps(x, w1, w5, w6, w7, w9):
    x = np.asarray(x, np.float32)
    assert x.shape[0] == N_CORES * NS
    shared = _prep_shared_inputs(w1, w5, w6, w7, w9)
    x16 = x.astype(np.float16).reshape(N_CORES, NS, G, 128, HW)
    return [
        {"x16": np.ascontiguousarray(x16[i]), **shared} for i in range(N_CORES)
    ]


def kernel(x, w1, w5, w6, w7, w9, _trace=False, _tmpdir=None):
    in_maps = _make_in_maps(x, w1, w5, w6, w7, w9)
    nc = _get_nc()
    res = run_bass_kernel_spmd(
        nc, in_maps, core_ids=list(range(N_CORES)), trace=_trace, tmpdir=_tmpdir
    )
    outs = [res.results[i]["out"] for i in range(N_CORES)]
    out = np.stack(outs, axis=0).reshape(N_CORES * NS, C, H, W)
    if _trace:
        kernel.last_exec_time_ns = res.exec_time_ns
        kernel.last_results = res
    return out


# revision 25
# speedup vs baseline: 1.0437x; 1.0437x over previous
"""Trainium2 Bass kernel v3h for the dense_cnn problem:

    t1 = conv1x1(x, w1); t2 = gelu(t1)
    t5 = dwconv5x5(t2, w5, pad=2)
    t6 = dwconv7x7_dil3(t5, w6, pad=9)
    t7 = conv1x1(t6, w7); t8 = t7 * t2; t9 = conv1x1(t8, w9)
    out = x + t9

Sharding: data-parallel over batch N=32 across 8 cores (4 samples/core).

Design (measured-cost driven):
  - PE matmuls sustain ~0.42ns/row back-to-back (LDWEIGHTS hidden), so the
    1x1 convs and dw7 run as diag matmuls.
  - dw5 runs on DVE/ACT/Pool chains (two independent tap-subchains per
    group); dw7 entirely on the PE as fp8e4 DoubleRow tap-pair diagonal
    matmuls (2 taps per 448-row pass).
  - v3h: the fp8 pair-diagonal weights for dw7 are precomputed on the
    HOST and DMA'd in (v3 built them with ~150 DVE ops serialized ahead
    of the first dw7 matmul).
  - PSUM allocated as 4-bank "wave" tiles [128, 4x512]; matmuls write
    448-elem chunks at 512 strides; evictions/elementwise consumers read
    the whole wave with one strided instruction.
  - x is cast to f16 on the host; one DMA in, reused by stage B and the
    residual add. Output DMA'd in 1792-elem waves.
"""

import numpy as np
import ml_dtypes

import concourse.bass as bass
import concourse.mybir as mybir
from concourse.tile import TileContext
from concourse.bass_utils import run_bass_kernel_spmd

# ---------------------------------------------------------------------------
# Workaround: this walrus build rejects >N sem waits on the TileContext tail
# drain ("Too many sync wait commands"). Split them one-per-drain.
from concourse.vector_clock import ScopedClock, VectorClock


def _drain_and_barrier_split(self, tick_clock, wait_clock):
    vc = tick_clock.global_clock
    for proc in range(len(vc)):
        tick = vc[proc]
        if tick <= 0:
            continue
        d = self.nc.sync.drain()
        req = ScopedClock({None: VectorClock([0] * len(vc))})
        req.require_at_least(None, proc, tick)
        wait_clock.add_sem_waits(d.ins, req)
    self.nc.all_engine_barrier()
    assert self.sems is not None
    popped = self.nc._tile_sem_poison_stack.pop()
    assert popped is self._sem_poison
    self.nc.clear_and_free_semaphores(list(self.sems.allocated().values()))
    self.nc.all_engine_barrier()


TileContext._drain_and_barrier = _drain_and_barrier_split

# This walrus build also rejects >1 sem wait on regular engine instructions.
# Post-process the serialized BIR: hoist excess waits onto same-engine NoOps
# inserted right before the instruction.
import json as _json

_orig_to_json_bytes = bass.Bass.to_json_bytes


def _to_json_bytes_split_waits(self):
    d = _json.loads(_orig_to_json_bytes(self))
    ctr = 0
    for fn in d.get("functions", []):
        for blk in fn.get("blocks", []):
            insts = blk.get("instructions", [])
            out = []
            for inst in insts:
                si = inst.get("sync_info")
                waits = (si or {}).get("on_wait") or []
                if len(waits) > 1:
                    for w in waits[:-1]:
                        out.append({
                            "debug": inst.get("debug", 0),
                            "engine": inst["engine"],
                            "ins": [],
                            "outs": [],
                            "name": f"{inst['name']}_hw{ctr}",
                            "opcode": "NoOp",
                            "sync_info": {"on_wait": [w], "on_update": []},
                        })
                        ctr += 1
                    si["on_wait"] = waits[-1:]
                out.append(inst)
            blk["instructions"] = out
    return _json.dumps(d).encode()


bass.Bass.to_json_bytes = _to_json_bytes_split_waits
# ---------------------------------------------------------------------------

F16 = mybir.dt.float16
F32 = mybir.dt.float32
AF = mybir.ActivationFunctionType
OP = mybir.AluOpType

N_CORES = 8
NS = 4              # samples per core
C, H, W = 384, 56, 56
G = 3               # channel groups of 128
HW = H * W          # 3136
W5P = 60            # t2 padded width/height (pad 2)
W7P = 74            # t5 padded width/height (pad 9)
W7PP = 80           # t5pad8 row pitch (32B-aligned partition pitch for PE fp8)
CH_ROWS = 8         # output rows per PSUM chunk
BANK = 512          # f32 elems per PSUM bank
CHF = CH_ROWS * W   # 448 elems per chunk

# dw5 runs entirely on DVE/ACT/Pool chains; dw7 entirely on the PE as
# fp8e4 DoubleRow tap-pair diagonal matmuls (2 taps per 448-row pass).
# Fraction of chain tap multiplies farmed to ACT (i%5 < ACT_OF_5).
ACT_OF_5 = 3
F8 = mybir.dt.float8e4
NP_F8 = ml_dtypes.float8_e4m3
PM = mybir.MatmulPerfMode

DW5_TAPS = [(dy, dx) for dy in range(5) for dx in range(5)]
DW7_TAPS = [(jy, jx) for jy in range(7) for jx in range(7)]
# dw7 tap pairs for DoubleRow (odd count: last pairs with a zero diagonal)
DW7_PAIRS = [(DW7_TAPS[2 * i], DW7_TAPS[2 * i + 1]) for i in range(24)] + [
    (DW7_TAPS[48], None)
]


def _chunks_of(rows):
    """Split `rows` output rows into PSUM chunks of <=8 rows."""
    out = []
    r = 0
    while r < rows:
        n = min(CH_ROWS, rows - r)
        out.append((r, n))
        r += n
    return out


def _waves_of(chunks):
    """Group chunk list into waves of up to 4 (one 4-bank PSUM tile)."""
    return [chunks[i : i + 4] for i in range(0, len(chunks), 4)]


def _build_program():
    nc = bass.Bass("TRN2", target_bir_lowering=False, debug=False)

    x_d = nc.dram_tensor("x16", [NS, G, 128, HW], F16, kind="ExternalInput")
    w1T_d = nc.dram_tensor("w1T", [G, 128, C], F16, kind="ExternalInput")
    w7T_d = nc.dram_tensor("w7T", [G, 128, C], F16, kind="ExternalInput")
    w9T_d = nc.dram_tensor("w9T", [G, 128, C], F16, kind="ExternalInput")
    w5t_d = nc.dram_tensor("w5t", [G, 128, 25], F32, kind="ExternalInput")
    dp6_d = nc.dram_tensor("dp6", [G, 25, 128, 256], F8, kind="ExternalInput")
    o_d = nc.dram_tensor("out", [NS, G, 128, HW], F32, kind="ExternalOutput")

    with TileContext(nc) as tc:
        with (
            tc.tile_pool(name="const", bufs=1) as const,
            tc.tile_pool(name="big16", bufs=6) as big16,
            tc.tile_pool(name="pads", bufs=1) as pads,
            tc.tile_pool(name="xload", bufs=3) as xload_p,
            tc.tile_pool(name="dve", bufs=1) as dve_p,
            tc.tile_pool(name="small", bufs=1) as small_p,
            tc.tile_pool(name="psum", bufs=2, space="PSUM") as pp,
        ):
            # ---- constants -------------------------------------------------
            w1T = [const.tile([128, C], F16, name=f"w1T{k}") for k in range(G)]
            w7T = [const.tile([128, C], F16, name=f"w7T{k}") for k in range(G)]
            w9T = [const.tile([128, C], F16, name=f"w9T{k}") for k in range(G)]
            w5t = [const.tile([128, 25], F32, name=f"w5t{g}") for g in range(G)]
            dpair6 = {}
            for k in range(G):
                nc.sync.dma_start(out=w1T[k][:], in_=w1T_d.ap()[k])
                nc.sync.dma_start(out=w7T[k][:], in_=w7T_d.ap()[k])
                nc.sync.dma_start(out=w9T[k][:], in_=w9T_d.ap()[k])
                nc.sync.dma_start(out=w5t[k][:], in_=w5t_d.ap()[k])
            # sample 0's x goes on the queue BEFORE the 75 dp6 descriptors
            # (those are only needed by D(0), ~300us in) so B(0) starts asap
            x16_pre = []
            for g in range(G):
                xl = xload_p.tile([128, HW], F16, name=f"xl0{g}", tag="xl")
                nc.sync.dma_start(out=xl[:], in_=x_d.ap()[0, g])
                x16_pre.append(xl)
            for k in range(G):
                for pi in range(len(DW7_PAIRS)):
                    t = const.tile([128, 2, 128], F8, name=f"dp6_{k}_{pi}")
                    nc.sync.dma_start(
                        out=t.rearrange("p a b -> p (a b)")[:],
                        in_=dp6_d.ap()[k, pi],
                    )
                    dpair6[(k, pi)] = t

            # ---- padded scratch (zero margins persist across samples) ------
            t2pad = [pads.tile([128, W5P * W5P], F16, name=f"t2p{g}") for g in range(G)]
            t5pad = [pads.tile([128, W7P * W7PP], F8, name=f"t5p{g}") for g in range(G)]
            for g in range(G):
                nc.gpsimd.memset(t2pad[g][:], 0.0)
                nc.gpsimd.memset(t5pad[g][:], 0.0)
            t2p3 = [t.rearrange("p (h w) -> p h w", w=W5P) for t in t2pad]
            t5p3 = [t.rearrange("p (h w) -> p h w", w=W7PP) for t in t5pad]

            env = dict(locals())
            for n in range(NS):
                _emit_sample(nc, env, n)
    return nc


def _psum_wave(env, n, tag, nch):
    """Allocate a PSUM tile of `nch` bank-aligned chunks."""
    pp = env["pp"]
    return pp.tile([128, nch, BANK], F32, name=f"pw{tag}{n}",
                   tag=f"pw{nch}", bufs=1)


import bass_rust as _br


def _dw_chain(nc, env, n, g, taps, wtile, ksz, src3, rows, r_start, dil,
              dst_view, stage, add_eng):
    """Two independent tap-subchains (A: even-indexed taps, B: odd) over rows
    [r_start, r_start+rows); final merge add writes dst_view. Per-tap mul on
    ACT or DVE; adds on `add_eng`."""
    dve_p = env["dve_p"]
    nel = rows * W
    tagsz = "h" if rows <= 28 else ""
    accs = []
    for h in range(2):
        a = dve_p.tile([128, nel], F16, name=f"acc{stage}{n}{g}{r_start}{h}",
                       tag=f"acc{tagsz}", bufs=4)
        accs.append(a.rearrange("p (h w) -> p h w", w=W))
    ntap = len(taps)
    started = [False, False]
    for i, tap in enumerate(taps):
        h = i % 2
        acc3 = accs[h]
        ty, tx = tap[0] * dil, tap[1] * dil
        src = src3[:, r_start + ty : r_start + ty + rows, tx : tx + W]
        sc = wtile[:, ksz * tap[0] + tap[1] : ksz * tap[0] + tap[1] + 1]
        if not started[h]:
            nc.vector.tensor_scalar_mul(acc3[:], src, sc)
            started[h] = True
            continue
        tmp = dve_p.tile([128, nel], F16, name=f"tmp{stage}{n}{g}{r_start}_{i}",
                         tag=f"tmp{tagsz}", bufs=4)
        tmp3 = tmp.rearrange("p (h w) -> p h w", w=W)
        if i % 5 < ACT_OF_5:
            nc.scalar.activation(tmp3[:], src, AF.Copy, scale=sc)
        else:
            nc.vector.tensor_scalar_mul(tmp3[:], src, sc)
        add_eng.tensor_tensor(acc3[:], acc3[:], tmp3[:], OP.add)
    add_eng.tensor_tensor(dst_view, accs[0][:], accs[1][:], OP.add)


def _pair_rhs(t5pad_g, pair, r0, nr):
    """Build the [128, 2, nr, 56] DoubleRow rhs AP for a dw7 tap pair."""
    ta, tb = pair
    off_a = (r0 + 3 * ta[0]) * W7PP + 3 * ta[1]
    if tb is None:
        delta = -3  # in-bounds dummy read, zero diagonal kills it
    else:
        off_b = (r0 + 3 * tb[0]) * W7PP + 3 * tb[1]
        delta = off_b - off_a
    base = t5pad_g[:, off_a : off_a + 1]
    ap = base.copy()
    ap.ap = _br.VecI64Pair(
        [[W7P * W7PP, 128], [delta, 2], [W7PP, nr], [1, W]]
    )
    return ap


def _dw7_pe(nc, env, n, g, dpair6, t5pad_g, t6g3):
    """dw7 entirely on PE: fp8 DoubleRow pair matmuls over all 7 chunks."""
    for wi, wave in enumerate(_waves_of(_chunks_of(H))):
        pw = _psum_wave(env, n, f"D{g}{wi}", len(wave))
        for pi in range(len(DW7_PAIRS)):
            for ci, (r0, nr) in enumerate(wave):
                nc.tensor.matmul(
                    pw[:, ci : ci + 1, 0 : nr * W],
                    dpair6[(g, pi)][:],
                    _pair_rhs(t5pad_g, DW7_PAIRS[pi], r0, nr),
                    start=(pi == 0),
                    stop=(pi == len(DW7_PAIRS) - 1),
                    perf_mode=PM.DoubleRow,
                )
        r0 = wave[0][0]
        rows = sum(nr for _, nr in wave)
        in_ap = pw[:, 0 : len(wave), 0 : CHF].rearrange(
            "p c (r w) -> p c r w", w=W
        )
        out_ap = t6g3[:, r0 : r0 + rows, :].rearrange(
            "p (c r) w -> p c r w", r=CH_ROWS
        )
        nc.scalar.activation(out_ap, in_ap, AF.Copy)


def _emit_sample(nc, env, n):
    x_d, o_d = env["x_d"], env["o_d"]
    w1T, w7T, w9T = env["w1T"], env["w7T"], env["w9T"]
    w5t = env["w5t"]
    dpair6, t5pad = env["dpair6"], env["t5pad"]
    t2p3, t5p3 = env["t2p3"], env["t5p3"]
    big16, xload_p, small_p = env["big16"], env["xload_p"], env["small_p"]

    ALL_CH = _chunks_of(H)  # 7 full chunks

    # A) load x16 (sample 0 was preloaded ahead of the dp6 descriptors)
    if n == 0:
        x16 = env["x16_pre"]
    else:
        x16 = []
        for g in range(G):
            xl = xload_p.tile([128, HW], F16, name=f"xl{n}{g}", tag="xl")
            nc.sync.dma_start(out=xl[:], in_=x_d.ap()[n, g])
            x16.append(xl)

    # B) t1 = w1 @ x16 ; t2 = gelu(t1) -> t2pad interior
    for m in range(G):
        for wi, wave in enumerate(_waves_of(ALL_CH)):
            pw = _psum_wave(env, n, f"B{m}{wi}", len(wave))
            for k in range(G):
                for ci, (r0, nr) in enumerate(wave):
                    nc.tensor.matmul(
                        pw[:, ci : ci + 1, 0 : nr * W],
                        w1T[k][:, 128 * m : 128 * (m + 1)],
                        x16[k][:, W * r0 : W * (r0 + nr)],
                        start=(k == 0),
                        stop=(k == G - 1),
                    )
            r0 = wave[0][0]
            rows = sum(nr for _, nr in wave)
            in_ap = pw[:, 0 : len(wave), 0 : CHF].rearrange(
                "p c (r w) -> p c r w", w=W
            )
            out_ap = t2p3[m][:, 2 + r0 : 2 + r0 + rows, 2 : 2 + W].rearrange(
                "p (c r) w -> p c r w", r=CH_ROWS
            )
            nc.scalar.activation(out_ap, in_ap, AF.Gelu)

    # C) t5 = dw5(t2) -> t5pad interior (fp8), no PE:
    #    g0, g1 full-image DVE chains; g2 split half DVE / half Pool.
    for g in range(2):
        _dw_chain(
            nc, env, n, g, DW5_TAPS, w5t[g], 5, t2p3[g], H, 0, 1,
            t5p3[g][:, 9 : 9 + H, 9 : 9 + W], "c", nc.vector,
        )
    HD = 42
    _dw_chain(
        nc, env, n, 2, DW5_TAPS, w5t[2], 5, t2p3[2], HD, 0, 1,
        t5p3[2][:, 9 : 9 + HD, 9 : 9 + W], "c", nc.vector,
    )
    _dw_chain(
        nc, env, n, 2, DW5_TAPS, w5t[2], 5, t2p3[2], H - HD, HD, 1,
        t5p3[2][:, 9 + HD : 9 + H, 9 : 9 + W], "cp", nc.gpsimd,
    )

    # D) t6 = dw7_dil3(t5) -> t6 (flat f16), entirely on PE via fp8 pairs
    t6 = []
    for g in range(G):
        t6g = big16.tile([128, HW], F16, name=f"t6_{n}_{g}", tag="b16")
        t6g3 = t6g.rearrange("p (h w) -> p h w", w=W)
        _dw7_pe(nc, env, n, g, dpair6, t5pad[g], t6g3)
        t6.append(t6g)

    # E) t7 = w7 @ t6 ; t8 = t7 * t2 (DVE tt from PSUM wave)
    t8 = []
    for m in range(G):
        t8m = big16.tile([128, HW], F16, name=f"t8_{n}_{m}", tag="b16")
        t8m3 = t8m.rearrange("p (h w) -> p h w", w=W)
        for wi, wave in enumerate(_waves_of(ALL_CH)):
            pw = _psum_wave(env, n, f"E{m}{wi}", len(wave))
            for k in range(G):
                for ci, (r0, nr) in enumerate(wave):
                    nc.tensor.matmul(
                        pw[:, ci : ci + 1, 0 : nr * W],
                        w7T[k][:, 128 * m : 128 * (m + 1)],
                        t6[k][:, W * r0 : W * (r0 + nr)],
                        start=(k == 0),
                        stop=(k == G - 1),
                    )
            r0 = wave[0][0]
            rows = sum(nr for _, nr in wave)
            ps_ap = pw[:, 0 : len(wave), 0 : CHF].rearrange(
                "p c (r w) -> p c r w", w=W
            )
            nc.vector.tensor_tensor(
                t8m3[:, r0 : r0 + rows, :].rearrange(
                    "p (c r) w -> p c r w", r=CH_ROWS
                ),
                ps_ap,
                t2p3[m][:, 2 + r0 : 2 + r0 + rows, 2 : 2 + W].rearrange(
                    "p (c r) w -> p c r w", r=CH_ROWS
                ),
                OP.mult,
            )
        t8.append(t8m)

    # F) t9 = w9 @ t8 ; out = x16 + t9 (DVE from PSUM wave) ; DMA out
    for m in range(G):
        for wi, wave in enumerate(_waves_of(ALL_CH)):
            pw = _psum_wave(env, n, f"F{m}{wi}", len(wave))
            for k in range(G):
                for ci, (r0, nr) in enumerate(wave):
                    nc.tensor.matmul(
                        pw[:, ci : ci + 1, 0 : nr * W],
                        w9T[k][:, 128 * m : 128 * (m + 1)],
                        t8[k][:, W * r0 : W * (r0 + nr)],
                        start=(k == 0),
                        stop=(k == G - 1),
                    )
            r0 = wave[0][0]
            nf = sum(nr for _, nr in wave) * W
            ost = small_p.tile([128, nf], F32, name=f"os{n}{m}{wi}",
                               tag=f"ost{wi}", bufs=2 if wi == 0 else 1)
            ps_ap = pw[:, 0 : len(wave), 0 : CHF]
            nc.vector.tensor_tensor(
                ost.rearrange("p (c f) -> p c f", f=CHF),
                ps_ap,
                x16[m][:, W * r0 : W * r0 + nf].rearrange(
                    "p (c f) -> p c f", f=CHF
                ),
                OP.add,
            )
            nc.sync.dma_start(
                out=o_d.ap()[n, m, :, W * r0 : W * r0 + nf], in_=ost[:]
            )


_NC_CACHE = None


def _get_nc():
    global _NC_CACHE
    if _NC_CACHE is None:
        _NC_CACHE = _build_program()
    return _NC_CACHE


def _prep_shared_inputs(w1, w5, w6, w7, w9):
    def lhsT(w):
        return (
            np.ascontiguousarray(np.asarray(w, np.float32).T)
            .astype(np.float16)
            .reshape(G, 128, C)
        )

    idx = np.arange(128)
    w6f = np.asarray(w6, np.float32).reshape(C, 49)
    dp6 = np.zeros((G, 25, 128, 256), NP_F8)
    for g in range(G):
        for pi, (ta, tb) in enumerate(DW7_PAIRS):
            blk = np.zeros((128, 256), np.float32)
            blk[idx, idx] = w6f[g * 128 : (g + 1) * 128, 7 * ta[0] + ta[1]]
            if tb is not None:
                blk[idx, 128 + idx] = w6f[g * 128 : (g + 1) * 128, 7 * tb[0] + tb[1]]
            dp6[g, pi] = blk.astype(NP_F8)

    return {
        "w1T": lhsT(w1),
        "w7T": lhsT(w7),
        "w9T": lhsT(w9),
        "w5t": np.asarray(w5, np.float32).reshape(C, 25).reshape(G, 128, 25),
        "dp6": dp6,
    }


def _make_in_maps(x, w1, w5, w6, w7, w9):
    x = np.asarray(x, np.float32)
    assert x.shape[0] == N_CORES * NS
    shared = _prep_shared_inputs(w1, w5, w6, w7, w9)
    x16 = x.astype(np.float16).reshape(N_CORES, NS, G, 128, HW)
    return [
        {"x16": np.ascontiguousarray(x16[i]), **shared} for i in range(N_CORES)
    ]


def kernel(x, w1, w5, w6, w7, w9, _trace=False, _tmpdir=None):
    in_maps = _make_in_maps(x, w1, w5, w6, w7, w9)
    nc = _get_nc()
    res = run_bass_kernel_spmd(
        nc, in_maps, core_ids=list(range(N_CORES)), trace=_trace, tmpdir=_tmpdir
    )
    outs = [res.results[i]["out"] for i in range(N_CORES)]
    out = np.stack(outs, axis=0).reshape(N_CORES * NS, C, H, W)
    if _trace:
        kernel.last_exec_time_ns = res.exec_time_ns
        kernel.last_results = res
    return out


# revision 26
# speedup vs baseline: 1.1796x; 1.1302x over previous
"""Trainium2 Bass kernel v3h for the dense_cnn problem:

    t1 = conv1x1(x, w1); t2 = gelu(t1)
    t5 = dwconv5x5(t2, w5, pad=2)
    t6 = dwconv7x7_dil3(t5, w6, pad=9)
    t7 = conv1x1(t6, w7); t8 = t7 * t2; t9 = conv1x1(t8, w9)
    out = x + t9

Sharding: data-parallel over batch N=32 across 8 cores (4 samples/core).

Design (measured-cost driven):
  - PE matmuls sustain ~0.42ns/row back-to-back (LDWEIGHTS hidden), so the
    1x1 convs and dw7 run as diag matmuls.
  - dw5 runs on DVE/ACT/Pool chains (two independent tap-subchains per
    group); dw7 entirely on the PE as fp8e4 DoubleRow tap-pair diagonal
    matmuls (2 taps per 448-row pass).
  - v3h: the fp8 pair-diagonal weights for dw7 are precomputed on the
    HOST and DMA'd in (v3 built them with ~150 DVE ops serialized ahead
    of the first dw7 matmul).
  - PSUM allocated as 4-bank "wave" tiles [128, 4x512]; matmuls write
    448-elem chunks at 512 strides; evictions/elementwise consumers read
    the whole wave with one strided instruction.
  - x is cast to f16 on the host; one DMA in, reused by stage B and the
    residual add. Output DMA'd in 1792-elem waves.
"""

import numpy as np
import ml_dtypes

import concourse.bass as bass
import concourse.mybir as mybir
from concourse.tile import TileContext
from concourse.bass_utils import run_bass_kernel_spmd

# ---------------------------------------------------------------------------
# Workaround: this walrus build rejects >N sem waits on the TileContext tail
# drain ("Too many sync wait commands"). Split them one-per-drain.
from concourse.vector_clock import ScopedClock, VectorClock


def _drain_and_barrier_split(self, tick_clock, wait_clock):
    vc = tick_clock.global_clock
    for proc in range(len(vc)):
        tick = vc[proc]
        if tick <= 0:
            continue
        d = self.nc.sync.drain()
        req = ScopedClock({None: VectorClock([0] * len(vc))})
        req.require_at_least(None, proc, tick)
        wait_clock.add_sem_waits(d.ins, req)
    self.nc.all_engine_barrier()
    assert self.sems is not None
    popped = self.nc._tile_sem_poison_stack.pop()
    assert popped is self._sem_poison
    self.nc.clear_and_free_semaphores(list(self.sems.allocated().values()))
    self.nc.all_engine_barrier()


TileContext._drain_and_barrier = _drain_and_barrier_split

# This walrus build also rejects >1 sem wait on regular engine instructions.
# Post-process the serialized BIR: hoist excess waits onto same-engine NoOps
# inserted right before the instruction.
import json as _json

_orig_to_json_bytes = bass.Bass.to_json_bytes


def _to_json_bytes_split_waits(self):
    d = _json.loads(_orig_to_json_bytes(self))
    ctr = 0
    for fn in d.get("functions", []):
        for blk in fn.get("blocks", []):
            insts = blk.get("instructions", [])
            out = []
            for inst in insts:
                si = inst.get("sync_info")
                waits = (si or {}).get("on_wait") or []
                if len(waits) > 1:
                    for w in waits[:-1]:
                        out.append({
                            "debug": inst.get("debug", 0),
                            "engine": inst["engine"],
                            "ins": [],
                            "outs": [],
                            "name": f"{inst['name']}_hw{ctr}",
                            "opcode": "NoOp",
                            "sync_info": {"on_wait": [w], "on_update": []},
                        })
                        ctr += 1
                    si["on_wait"] = waits[-1:]
                out.append(inst)
            blk["instructions"] = out
    return _json.dumps(d).encode()


bass.Bass.to_json_bytes = _to_json_bytes_split_waits
# ---------------------------------------------------------------------------

F16 = mybir.dt.float16
F32 = mybir.dt.float32
AF = mybir.ActivationFunctionType
OP = mybir.AluOpType

N_CORES = 8
NS = 4              # samples per core
C, H, W = 384, 56, 56
G = 3               # channel groups of 128
HW = H * W          # 3136
W5P = 60            # t2 padded width/height (pad 2)
W7P = 74            # t5 padded width/height (pad 9)
W7PP = 80           # t5pad8 row pitch (32B-aligned partition pitch for PE fp8)
CH_ROWS = 8         # output rows per PSUM chunk
BANK = 512          # f32 elems per PSUM bank
CHF = CH_ROWS * W   # 448 elems per chunk

# dw5 runs entirely on DVE/ACT/Pool chains; dw7 entirely on the PE as
# fp8e4 DoubleRow tap-pair diagonal matmuls (2 taps per 448-row pass).
# Fraction of chain tap multiplies farmed to ACT (i%5 < ACT_OF_5).
ACT_OF_5 = 3
F8 = mybir.dt.float8e4
NP_F8 = ml_dtypes.float8_e4m3
PM = mybir.MatmulPerfMode

DW5_TAPS = [(dy, dx) for dy in range(5) for dx in range(5)]
DW7_TAPS = [(jy, jx) for jy in range(7) for jx in range(7)]
# dw7 tap pairs for DoubleRow (odd count: last pairs with a zero diagonal)
DW7_PAIRS = [(DW7_TAPS[2 * i], DW7_TAPS[2 * i + 1]) for i in range(24)] + [
    (DW7_TAPS[48], None)
]


def _chunks_of(rows):
    """Split `rows` output rows into PSUM chunks of <=8 rows."""
    out = []
    r = 0
    while r < rows:
        n = min(CH_ROWS, rows - r)
        out.append((r, n))
        r += n
    return out


def _waves_of(chunks):
    """Group chunk list into waves of up to 4 (one 4-bank PSUM tile)."""
    return [chunks[i : i + 4] for i in range(0, len(chunks), 4)]


def _build_program():
    nc = bass.Bass("TRN2", target_bir_lowering=False, debug=False)

    x_d = nc.dram_tensor("x16", [NS, G, 128, HW], F16, kind="ExternalInput")
    w1T_d = nc.dram_tensor("w1T", [G, 128, C], F16, kind="ExternalInput")
    w7T_d = nc.dram_tensor("w7T", [G, 128, C], F16, kind="ExternalInput")
    w9T_d = nc.dram_tensor("w9T", [G, 128, C], F16, kind="ExternalInput")
    w5t_d = nc.dram_tensor("w5t", [G, 128, 25], F32, kind="ExternalInput")
    w6t_d = nc.dram_tensor("w6t", [G, 128, 49], F32, kind="ExternalInput")
    id_d = nc.dram_tensor("ident", [128, 128], F16, kind="ExternalInput")
    o_d = nc.dram_tensor("out", [NS, G, 128, HW], F32, kind="ExternalOutput")

    with TileContext(nc) as tc:
        with (
            tc.tile_pool(name="const", bufs=1) as const,
            tc.tile_pool(name="big16", bufs=6) as big16,
            tc.tile_pool(name="pads", bufs=1) as pads,
            tc.tile_pool(name="xload", bufs=3) as xload_p,
            tc.tile_pool(name="dve", bufs=1) as dve_p,
            tc.tile_pool(name="small", bufs=1) as small_p,
            tc.tile_pool(name="psum", bufs=2, space="PSUM") as pp,
        ):
            # ---- constants -------------------------------------------------
            w1T = [const.tile([128, C], F16, name=f"w1T{k}") for k in range(G)]
            w7T = [const.tile([128, C], F16, name=f"w7T{k}") for k in range(G)]
            w9T = [const.tile([128, C], F16, name=f"w9T{k}") for k in range(G)]
            w5t = [const.tile([128, 25], F32, name=f"w5t{g}") for g in range(G)]
            w6t = [const.tile([128, 49], F32, name=f"w6t{g}") for g in range(G)]
            ident = const.tile([128, 128], F16, name="ident")
            for k in range(G):
                nc.sync.dma_start(out=w1T[k][:], in_=w1T_d.ap()[k])
                nc.sync.dma_start(out=w7T[k][:], in_=w7T_d.ap()[k])
                nc.sync.dma_start(out=w9T[k][:], in_=w9T_d.ap()[k])
                nc.sync.dma_start(out=w5t[k][:], in_=w5t_d.ap()[k])
                nc.sync.dma_start(out=w6t[k][:], in_=w6t_d.ap()[k])
            nc.sync.dma_start(out=ident[:], in_=id_d.ap())

            # fp8 DoubleRow pair-diagonal weights for dw7: [128, 2, 128],
            # k-tile t holds diag(w6[:, pair[t]]) (zero diag for the odd tail).
            dpair6 = {}
            for g in range(G):
                for pi, (ta, tb) in enumerate(DW7_PAIRS):
                    t = const.tile([128, 2, 128], F8, name=f"dp6_{g}_{pi}")
                    t2d = t.rearrange("p a b -> p (a b)")
                    ka = 7 * ta[0] + ta[1]
                    nc.vector.tensor_scalar_mul(
                        t2d[:, 0:128], ident[:], w6t[g][:, ka : ka + 1]
                    )
                    if tb is None:
                        nc.vector.memset(t2d[:, 128:256], 0.0)
                    else:
                        kb = 7 * tb[0] + tb[1]
                        nc.vector.tensor_scalar_mul(
                            t2d[:, 128:256], ident[:], w6t[g][:, kb : kb + 1]
                        )
                    dpair6[(g, pi)] = t

            # ---- padded scratch (zero margins persist across samples) ------
            t2pad = [pads.tile([128, W5P * W5P], F16, name=f"t2p{g}") for g in range(G)]
            t5pad = [pads.tile([128, W7P * W7PP], F8, name=f"t5p{g}") for g in range(G)]
            for g in range(G):
                nc.gpsimd.memset(t2pad[g][:], 0.0)
                nc.gpsimd.memset(t5pad[g][:], 0.0)
            t2p3 = [t.rearrange("p (h w) -> p h w", w=W5P) for t in t2pad]
            t5p3 = [t.rearrange("p (h w) -> p h w", w=W7PP) for t in t5pad]

            env = dict(locals())
            for n in range(NS):
                _emit_sample(nc, env, n)
    return nc


def _psum_wave(env, n, tag, nch):
    """Allocate a PSUM tile of `nch` bank-aligned chunks."""
    pp = env["pp"]
    return pp.tile([128, nch, BANK], F32, name=f"pw{tag}{n}",
                   tag=f"pw{nch}", bufs=1)


import bass_rust as _br


def _dw_chain(nc, env, n, g, taps, wtile, ksz, src3, rows, r_start, dil,
              dst_view, stage, add_eng):
    """Two independent tap-subchains (A: even-indexed taps, B: odd) over rows
    [r_start, r_start+rows); final merge add writes dst_view. Per-tap mul on
    ACT or DVE; adds on `add_eng`."""
    dve_p = env["dve_p"]
    nel = rows * W
    tagsz = "h" if rows <= 28 else ""
    accs = []
    for h in range(2):
        a = dve_p.tile([128, nel], F16, name=f"acc{stage}{n}{g}{r_start}{h}",
                       tag=f"acc{tagsz}", bufs=4)
        accs.append(a.rearrange("p (h w) -> p h w", w=W))
    ntap = len(taps)
    started = [False, False]
    for i, tap in enumerate(taps):
        h = i % 2
        acc3 = accs[h]
        ty, tx = tap[0] * dil, tap[1] * dil
        src = src3[:, r_start + ty : r_start + ty + rows, tx : tx + W]
        sc = wtile[:, ksz * tap[0] + tap[1] : ksz * tap[0] + tap[1] + 1]
        if not started[h]:
            nc.vector.tensor_scalar_mul(acc3[:], src, sc)
            started[h] = True
            continue
        tmp = dve_p.tile([128, nel], F16, name=f"tmp{stage}{n}{g}{r_start}_{i}",
                         tag=f"tmp{tagsz}", bufs=4)
        tmp3 = tmp.rearrange("p (h w) -> p h w", w=W)
        if i % 5 < ACT_OF_5:
            nc.scalar.activation(tmp3[:], src, AF.Copy, scale=sc)
        else:
            nc.vector.tensor_scalar_mul(tmp3[:], src, sc)
        add_eng.tensor_tensor(acc3[:], acc3[:], tmp3[:], OP.add)
    add_eng.tensor_tensor(dst_view, accs[0][:], accs[1][:], OP.add)


def _pair_rhs(t5pad_g, pair, r0, nr):
    """Build the [128, 2, nr, 56] DoubleRow rhs AP for a dw7 tap pair."""
    ta, tb = pair
    off_a = (r0 + 3 * ta[0]) * W7PP + 3 * ta[1]
    if tb is None:
        delta = -3  # in-bounds dummy read, zero diagonal kills it
    else:
        off_b = (r0 + 3 * tb[0]) * W7PP + 3 * tb[1]
        delta = off_b - off_a
    base = t5pad_g[:, off_a : off_a + 1]
    ap = base.copy()
    ap.ap = _br.VecI64Pair(
        [[W7P * W7PP, 128], [delta, 2], [W7PP, nr], [1, W]]
    )
    return ap


def _dw7_pe(nc, env, n, g, dpair6, t5pad_g, t6g3):
    """dw7 entirely on PE: fp8 DoubleRow pair matmuls over all 7 chunks."""
    for wi, wave in enumerate(_waves_of(_chunks_of(H))):
        pw = _psum_wave(env, n, f"D{g}{wi}", len(wave))
        for pi in range(len(DW7_PAIRS)):
            for ci, (r0, nr) in enumerate(wave):
                nc.tensor.matmul(
                    pw[:, ci : ci + 1, 0 : nr * W],
                    dpair6[(g, pi)][:],
                    _pair_rhs(t5pad_g, DW7_PAIRS[pi], r0, nr),
                    start=(pi == 0),
                    stop=(pi == len(DW7_PAIRS) - 1),
                    perf_mode=PM.DoubleRow,
                )
        r0 = wave[0][0]
        rows = sum(nr for _, nr in wave)
        in_ap = pw[:, 0 : len(wave), 0 : CHF].rearrange(
            "p c (r w) -> p c r w", w=W
        )
        out_ap = t6g3[:, r0 : r0 + rows, :].rearrange(
            "p (c r) w -> p c r w", r=CH_ROWS
        )
        nc.scalar.activation(out_ap, in_ap, AF.Copy)


def _emit_sample(nc, env, n):
    x_d, o_d = env["x_d"], env["o_d"]
    w1T, w7T, w9T = env["w1T"], env["w7T"], env["w9T"]
    w5t = env["w5t"]
    dpair6, t5pad = env["dpair6"], env["t5pad"]
    t2p3, t5p3 = env["t2p3"], env["t5p3"]
    big16, xload_p, small_p = env["big16"], env["xload_p"], env["small_p"]

    ALL_CH = _chunks_of(H)  # 7 full chunks

    # A) load x16
    x16 = []
    for g in range(G):
        xl = xload_p.tile([128, HW], F16, name=f"xl{n}{g}", tag="xl")
        nc.sync.dma_start(out=xl[:], in_=x_d.ap()[n, g])
        x16.append(xl)

    # B) t1 = w1 @ x16 ; t2 = gelu(t1) -> t2pad interior
    for m in range(G):
        for wi, wave in enumerate(_waves_of(ALL_CH)):
            pw = _psum_wave(env, n, f"B{m}{wi}", len(wave))
            for k in range(G):
                for ci, (r0, nr) in enumerate(wave):
                    nc.tensor.matmul(
                        pw[:, ci : ci + 1, 0 : nr * W],
                        w1T[k][:, 128 * m : 128 * (m + 1)],
                        x16[k][:, W * r0 : W * (r0 + nr)],
                        start=(k == 0),
                        stop=(k == G - 1),
                    )
            r0 = wave[0][0]
            rows = sum(nr for _, nr in wave)
            in_ap = pw[:, 0 : len(wave), 0 : CHF].rearrange(
                "p c (r w) -> p c r w", w=W
            )
            out_ap = t2p3[m][:, 2 + r0 : 2 + r0 + rows, 2 : 2 + W].rearrange(
                "p (c r) w -> p c r w", r=CH_ROWS
            )
            nc.scalar.activation(out_ap, in_ap, AF.Gelu)

    # C) t5 = dw5(t2) -> t5pad interior (fp8), no PE:
    #    g0, g1 full-image DVE chains; g2 split half DVE / half Pool.
    for g in range(2):
        _dw_chain(
            nc, env, n, g, DW5_TAPS, w5t[g], 5, t2p3[g], H, 0, 1,
            t5p3[g][:, 9 : 9 + H, 9 : 9 + W], "c", nc.vector,
        )
    HD = 42
    _dw_chain(
        nc, env, n, 2, DW5_TAPS, w5t[2], 5, t2p3[2], HD, 0, 1,
        t5p3[2][:, 9 : 9 + HD, 9 : 9 + W], "c", nc.vector,
    )
    _dw_chain(
        nc, env, n, 2, DW5_TAPS, w5t[2], 5, t2p3[2], H - HD, HD, 1,
        t5p3[2][:, 9 + HD : 9 + H, 9 : 9 + W], "cp", nc.gpsimd,
    )

    # D) t6 = dw7_dil3(t5) -> t6 (flat f16), entirely on PE via fp8 pairs
    t6 = []
    for g in range(G):
        t6g = big16.tile([128, HW], F16, name=f"t6_{n}_{g}", tag="b16")
        t6g3 = t6g.rearrange("p (h w) -> p h w", w=W)
        _dw7_pe(nc, env, n, g, dpair6, t5pad[g], t6g3)
        t6.append(t6g)

    # E) t7 = w7 @ t6 ; t8 = t7 * t2 (DVE tt from PSUM wave)
    t8 = []
    for m in range(G):
        t8m = big16.tile([128, HW], F16, name=f"t8_{n}_{m}", tag="b16")
        t8m3 = t8m.rearrange("p (h w) -> p h w", w=W)
        for wi, wave in enumerate(_waves_of(ALL_CH)):
            pw = _psum_wave(env, n, f"E{m}{wi}", len(wave))
            for k in range(G):
                for ci, (r0, nr) in enumerate(wave):
                    nc.tensor.matmul(
                        pw[:, ci : ci + 1, 0 : nr * W],
                        w7T[k][:, 128 * m : 128 * (m + 1)],
                        t6[k][:, W * r0 : W * (r0 + nr)],
                        start=(k == 0),
                        stop=(k == G - 1),
                    )
            r0 = wave[0][0]
            rows = sum(nr for _, nr in wave)
            ps_ap = pw[:, 0 : len(wave), 0 : CHF].rearrange(
                "p c (r w) -> p c r w", w=W
            )
            nc.vector.tensor_tensor(
                t8m3[:, r0 : r0 + rows, :].rearrange(
                    "p (c r) w -> p c r w", r=CH_ROWS
                ),
                ps_ap,
                t2p3[m][:, 2 + r0 : 2 + r0 + rows, 2 : 2 + W].rearrange(
                    "p (c r) w -> p c r w", r=CH_ROWS
                ),
                OP.mult,
            )
        t8.append(t8m)

    # F) t9 = w9 @ t8 ; out = x16 + t9 (DVE from PSUM wave) ; DMA out
    for m in range(G):
        for wi, wave in enumerate(_waves_of(ALL_CH)):
            pw = _psum_wave(env, n, f"F{m}{wi}", len(wave))
            for k in range(G):
                for ci, (r0, nr) in enumerate(wave):
                    nc.tensor.matmul(
                        pw[:, ci : ci + 1, 0 : nr * W],
                        w9T[k][:, 128 * m : 128 * (m + 1)],
                        t8[k][:, W * r0 : W * (r0 + nr)],
                        start=(k == 0),
                        stop=(k == G - 1),
                    )
            r0 = wave[0][0]
            nf = sum(nr for _, nr in wave) * W
            ost = small_p.tile([128, nf], F32, name=f"os{n}{m}{wi}",
                               tag=f"ost{wi}", bufs=2 if wi == 0 else 1)
            ps_ap = pw[:, 0 : len(wave), 0 : CHF]
            nc.vector.tensor_tensor(
                ost.rearrange("p (c f) -> p c f", f=CHF),
                ps_ap,
                x16[m][:, W * r0 : W * r0 + nf].rearrange(
                    "p (c f) -> p c f", f=CHF
                ),
                OP.add,
            )
            nc.sync.dma_start(
                out=o_d.ap()[n, m, :, W * r0 : W * r0 + nf], in_=ost[:]
            )


_NC_CACHE = None


def _get_nc():
    global _NC_CACHE
    if _NC_CACHE is None:
        _NC_CACHE = _build_program()
    return _NC_CACHE


def _prep_shared_inputs(w1, w5, w6, w7, w9):
    def lhsT(w):
        return (
            np.ascontiguousarray(np.asarray(w, np.float32).T)
            .astype(np.float16)
            .reshape(G, 128, C)
        )

    return {
        "w1T": lhsT(w1),
        "w7T": lhsT(w7),
        "w9T": lhsT(w9),
        "w5t": np.asarray(w5, np.float32).reshape(C, 25).reshape(G, 128, 25),
        "w6t": np.asarray(w6, np.float32).reshape(C, 49).reshape(G, 128, 49),
        "ident": np.eye(128, dtype=np.float16),
    }


def _make_in_maps(x, w1, w5, w6, w7, w9):
    x = np.asarray(x, np.float32)
    assert x.shape[0] == N_CORES * NS
    shared = _prep_shared_inputs(w1, w5, w6, w7, w9)
    x16 = x.astype(np.float16).reshape(N_CORES, NS, G, 128, HW)
    return [
        {"x16": np.ascontiguousarray(x16[i]), **shared} for i in range(N_CORES)
    ]


def kernel(x, w1, w5, w6, w7, w9, _trace=False, _tmpdir=None):
    in_maps = _make_in_maps(x, w1, w5, w6, w7, w9)
    nc = _get_nc()
    res = run_bass_kernel_spmd(
        nc, in_maps, core_ids=list(range(N_CORES)), trace=_trace, tmpdir=_tmpdir
    )
    outs = [res.results[i]["out"] for i in range(N_CORES)]
    out = np.stack(outs, axis=0).reshape(N_CORES * NS, C, H, W)
    if _trace:
        kernel.last_exec_time_ns = res.exec_time_ns
        kernel.last_results = res
    return out
